# revision 9
# baseline (speedup 1.0000x reference)
import sys

if "/opt/trn_rl_repo" not in sys.path:
    sys.path.insert(0, "/opt/trn_rl_repo")

import numpy as np

import concourse.bass as bass
import concourse.tile as tile
from concourse import bacc, mybir
from concourse.masks import make_identity

# Problem shapes (hardcoded per spec)
B, T, D, NU, V = 32, 128, 512, 1024, 32000
G4 = 4 * NU
NCORES = 8
VS = V // NCORES  # vocab shard per core

F32 = mybir.dt.float32
BF16 = mybir.dt.bfloat16

# col-group order (i, f, o, c) -> source gate index in U/b/mask order (i, f, c, o)
CG_SRC = [0, 1, 3, 2]

UK = NU // 128  # 8 u k-tiles
DK = D // 128   # 4 d k-tiles
NCH = 2         # 512-wide halves of the 1024 unit dim


def _ap(apobj, dims, extra_offset=0):
    return bass.AP(tensor=apobj.tensor, offset=apobj.offset + extra_offset, ap=dims)


def build_kernel(t_steps=T, with_bias=False, with_bd=False):
    nc = bacc.Bacc("TRN2", target_bir_lowering=False, debug=False)

    x_d = nc.dram_tensor("x", [B, T, D], F32, kind="ExternalInput")
    h0_d = nc.dram_tensor("h0", [B, NU], F32, kind="ExternalInput")
    c0_d = nc.dram_tensor("c0", [B, NU], F32, kind="ExternalInput")
    w_d = nc.dram_tensor("W", [D, G4], F32, kind="ExternalInput")
    u_d = nc.dram_tensor("U", [NU, G4], F32, kind="ExternalInput")
    b_d = nc.dram_tensor("b", [G4], F32, kind="ExternalInput")
    wd_d = nc.dram_tensor("Wd", [NU, VS], F32, kind="ExternalInput")
    bd_d = nc.dram_tensor("bd", [VS], F32, kind="ExternalInput")
    m_d = nc.dram_tensor("rm", [4, B, NU], F32, kind="ExternalInput")

    logits_d = nc.dram_tensor("logits", [B, T, VS], F32, kind="ExternalOutput")
    ht_d = nc.dram_tensor("hT", [B, NU], F32, kind="ExternalOutput")
    ct_d = nc.dram_tensor("cT", [B, NU], F32, kind="ExternalOutput")

    with tile.TileContext(nc) as tc:
        with tc.tile_pool(name="const", bufs=1) as const:
            # persistent SBUF tensors
            hsT = const.tile([128, UK, B * T], BF16)     # h history, u-land, col = b*T + t
            mT = const.tile([128, UK, 4, B], BF16)       # masks, u-land, cg order
            h0T = const.tile([128, UK, B], BF16)
            idf = const.tile([128, 128], F32)
            idb = const.tile([128, 128], BF16)
            make_identity(nc, idf[:, :])
            make_identity(nc, idb[:, :])
            if with_bias:
                bz = const.tile([1, 4, NU], BF16)
                ones = const.tile([1, B], BF16)
                nc.vector.memset(ones[:, :], 1.0)
            if t_steps < T:
                nc.vector.memset(hsT[:, :, :], 0.0)

            with tc.tile_pool(name="wu", bufs=1) as wu, \
                 tc.tile_pool(name="stag", bufs=2) as stag:
                ppre_cm = tc.tile_pool(name="ppre", bufs=2, space="PSUM")
                ppre = ppre_cm.__enter__()
                u_sb = wu.tile([128, UK, G4], BF16)
                w_sb = wu.tile([128, DK, G4], BF16)

                # ---- load + cast U, W ----
                for uk in range(UK):
                    for hh in range(4):
                        sg = stag.tile([128, 1024], F32, tag="stag")
                        nc.sync.dma_start(out=sg[:, :], in_=u_d[uk * 128:(uk + 1) * 128, hh * 1024:(hh + 1) * 1024])
                        nc.vector.tensor_copy(out=u_sb[:, uk, hh * 1024:(hh + 1) * 1024], in_=sg[:, :])
                for dk in range(DK):
                    for hh in range(4):
                        sg = stag.tile([128, 1024], F32, tag="stag")
                        nc.sync.dma_start(out=sg[:, :], in_=w_d[dk * 128:(dk + 1) * 128, hh * 1024:(hh + 1) * 1024])
                        nc.vector.tensor_copy(out=w_sb[:, dk, hh * 1024:(hh + 1) * 1024], in_=sg[:, :])
                if with_bias:
                    for sg_i in range(4):
                        sg = stag.tile([128, 1024], F32, tag="stag")
                        nc.sync.dma_start(out=sg[0:1, 0:NU], in_=_ap(b_d[:], [[0, 1], [1, NU]], extra_offset=sg_i * NU))
                        g = CG_SRC.index(sg_i)
                        nc.vector.tensor_copy(out=bz[:, g, :], in_=sg[0:1, 0:NU])

                # ---- masks -> u-land bf16 ----
                for g in range(4):
                    sg = stag.tile([128, 1024], F32, tag="stagm")
                    nc.sync.dma_start(out=sg[0:B, 0:NU], in_=m_d[CG_SRC[g], :, :])
                    mb = stag.tile([B, NU], BF16, tag="mb")
                    nc.vector.tensor_copy(out=mb[:, :], in_=sg[0:B, 0:NU])
                    pt = ppre.tile([128, UK, B], BF16, tag="pt")
                    for uk in range(UK):
                        nc.tensor.transpose(pt[:, uk, :], mb[:, uk * 128:(uk + 1) * 128], idb[0:B, 0:B])
                    nc.vector.tensor_copy(out=mT[:, :, g, :], in_=pt[:, :, :])

                # ---- h0 -> u-land bf16 ----
                sg = stag.tile([128, 1024], F32, tag="stagm")
                nc.sync.dma_start(out=sg[0:B, 0:NU], in_=h0_d[:, :])
                hb = stag.tile([B, NU], BF16, tag="mb")
                nc.vector.tensor_copy(out=hb[:, :], in_=sg[0:B, 0:NU])
                pt = ppre.tile([128, UK, B], BF16, tag="pt")
                for uk in range(UK):
                    nc.tensor.transpose(pt[:, uk, :], hb[:, uk * 128:(uk + 1) * 128], idb[0:B, 0:B])
                nc.vector.tensor_copy(out=h0T[:, :, :], in_=pt[:, :, :])

                ppre_cm.__exit__(None, None, None)

                # ---- c0 -> u-land f32 (kept in ping-pong pool) ----
                with tc.tile_pool(name="state", bufs=2) as state, \
                     tc.tile_pool(name="step", bufs=2) as step, \
                     tc.tile_pool(name="xs", bufs=3) as xs, \
                     tc.tile_pool(name="pz", bufs=2, space="PSUM") as pz, \
                     tc.tile_pool(name="pT", bufs=2, space="PSUM") as pT, \
                     tc.tile_pool(name="px", bufs=2, space="PSUM") as px:

                    sgc = stag.tile([128, 1024], F32, tag="stagm")
                    nc.sync.dma_start(out=sgc[0:B, 0:NU], in_=c0_d[:, :])
                    c_cur = state.tile([128, UK, B], F32, tag="c")
                    ptc = px.tile([128, UK, B], F32, tag="ptx")
                    for uk in range(UK):
                        nc.tensor.transpose(ptc[:, uk, :], sgc[0:B, uk * 128:(uk + 1) * 128], idf[0:B, 0:B])
                    nc.vector.tensor_copy(out=c_cur[:, :, :], in_=ptc[:, :, :])

                    # hm for t=0 from h0T
                    hm_cur = step.tile([128, UK, 4 * B], BF16, tag="hm")
                    h0_bc = _ap(h0T[:, :, :], [h0T[:, :, :].ap[0], h0T[:, :, :].ap[1], [0, 4], h0T[:, :, :].ap[2]])
                    nc.vector.tensor_mul(out=hm_cur[:, :, :], in0=h0_bc, in1=mT[:, :, :, :])

                    # x for t=0
                    def emit_x(t):
                        xst = xs.tile([B, D], F32, tag="xst")
                        nc.sync.dma_start(out=xst[:, :], in_=x_d[:, t, :])
                        pxt = px.tile([128, DK, B], F32, tag="ptx")
                        for dk in range(DK):
                            nc.tensor.transpose(pxt[:, dk, :], xst[:, dk * 128:(dk + 1) * 128], idf[0:B, 0:B])
                        xt4 = step.tile([128, DK, B], BF16, tag="xt4")
                        nc.vector.tensor_copy(out=xt4[:, :, :], in_=pxt[:, :, :])
                        return xt4

                    xt4_cur = emit_x(0)

                    hs_full = hsT[:, :, :]

                    for t in range(t_steps):
                        # ---- z matmuls: psum [(4g x 32b), 512] x 2 halves ----
                        zps = [pz.tile([128, 512], F32, tag=f"z{i}", name=f"z{i}") for i in range(NCH)]
                        for nch in range(NCH):
                            for g in range(4):
                                col0 = CG_SRC[g] * NU + nch * 512
                                for dk in range(DK):
                                    nc.tensor.matmul(
                                        zps[nch][32 * g:32 * (g + 1), :],
                                        xt4_cur[:, dk, :],
                                        w_sb[:, dk, col0:col0 + 512],
                                        start=(dk == 0), stop=False,
                                        tile_position=(0, 32 * g))
                                for uk in range(UK):
                                    last = (uk == UK - 1) and not with_bias
                                    nc.tensor.matmul(
                                        zps[nch][32 * g:32 * (g + 1), :],
                                        hm_cur[:, uk, 32 * g:32 * (g + 1)],
                                        u_sb[:, uk, col0:col0 + 512],
                                        start=False, stop=last,
                                        tile_position=(0, 32 * g))
                                if with_bias:
                                    nc.tensor.matmul(
                                        zps[nch][32 * g:32 * (g + 1), :],
                                        ones[0:1, 0:32],
                                        bz[:, CG_SRC[g], nch * 512:(nch + 1) * 512],
                                        start=False, stop=True,
                                        tile_position=(0, 32 * g))

                        # x transposes for next step (fills PE while ACT runs)
                        if t + 1 < t_steps:
                            xt4_nxt = emit_x(t + 1)

                        # ---- gates: sigmoid(i,f,o), copy zc; bf16 ----
                        gates = step.tile([128, NU], BF16, tag="gates")
                        for nch in range(NCH):
                            nc.scalar.activation(out=gates[0:96, nch * 512:(nch + 1) * 512],
                                                 in_=zps[nch][0:96, :],
                                                 func=mybir.ActivationFunctionType.Sigmoid)
                            nc.vector.tensor_copy(out=gates[96:128, nch * 512:(nch + 1) * 512],
                                                  in_=zps[nch][96:128, :])

                        # ---- transpose gates to u-land ----
                        gT = pT.tile([128, UK, 128], BF16, tag="gT")
                        for uk in range(UK):
                            nc.tensor.transpose(gT[:, uk, :], gates[:, uk * 128:(uk + 1) * 128], idb[:, :])

                        # ---- cell math in u-land ----
                        tcn = step.tile([128, UK, B], F32, tag="tc", bufs=1)
                        nc.scalar.activation(out=tcn[:, :, :], in_=gT[:, :, 96:128],
                                             func=mybir.ActivationFunctionType.Tanh)
                        fc = step.tile([128, UK, B], F32, tag="fc", bufs=1)
                        nc.vector.tensor_mul(out=fc[:, :, :], in0=gT[:, :, 32:64], in1=c_cur[:, :, :])
                        ic = step.tile([128, UK, B], F32, tag="ic", bufs=1)
                        nc.vector.tensor_mul(out=ic[:, :, :], in0=gT[:, :, 0:32], in1=tcn[:, :, :])
                        c_nxt = state.tile([128, UK, B], F32, tag="c")
                        nc.vector.tensor_add(out=c_nxt[:, :, :], in0=fc[:, :, :], in1=ic[:, :, :])
                        th = step.tile([128, UK, B], F32, tag="th", bufs=1)
                        nc.scalar.activation(out=th[:, :, :], in_=c_nxt[:, :, :],
                                             func=mybir.ActivationFunctionType.Tanh)
                        # h (bf16) directly into history: cols {b*T + t}
                        hdst = _ap(hs_full, [hs_full.ap[0], hs_full.ap[1], [T, B]], extra_offset=t)
                        nc.vector.tensor_mul(out=hdst, in0=gT[:, :, 64:96], in1=th[:, :, :])

                        # hm for next step
                        if t + 1 < t_steps:
                            hm_nxt = step.tile([128, UK, 4 * B], BF16, tag="hm")
                            hsrc = _ap(hs_full, [hs_full.ap[0], hs_full.ap[1], [0, 4], [T, B]], extra_offset=t)
                            nc.vector.tensor_mul(out=hm_nxt[:, :, :], in0=hsrc, in1=mT[:, :, :, :])
                            hm_cur = hm_nxt
                            xt4_cur = xt4_nxt
                        c_cur = c_nxt

                    # ---- final hT (from bf16 h) and cT (f32) back to b-land ----
                    hlast = _ap(hs_full, [hs_full.ap[0], hs_full.ap[1], [T, B]], extra_offset=t_steps - 1)
                    hlast_bf = step.tile([128, UK, B], BF16, tag="hlast")
                    nc.vector.tensor_copy(out=hlast_bf[:, :, :], in_=hlast)
                    for uk in range(UK):
                        pbt = pT.tile([B, 128], BF16, tag="gT", name="pbt")
                        nc.tensor.transpose(pbt[:, :], hlast_bf[:, uk, :], idb[:, :])
                        hseg = step.tile([B, 128], F32, tag="hseg", name="hseg", bufs=2)
                        nc.vector.tensor_copy(out=hseg[:, :], in_=pbt[:, :])
                        nc.gpsimd.dma_start(out=ht_d[:, uk * 128:(uk + 1) * 128], in_=hseg[:, :])
                        pbc = px.tile([B, 128], F32, tag="ptx", name="pbc")
                        cl = step.tile([128, B], F32, tag="cl", bufs=2)
                        nc.vector.tensor_copy(out=cl[:, :], in_=c_cur[:, uk, :])
                        nc.tensor.transpose(pbc[:, :], cl[:, :], idf[:, :])
                        cseg = step.tile([B, 128], F32, tag="cseg", name="cseg", bufs=2)
                        nc.vector.tensor_copy(out=cseg[:, :], in_=pbc[:, :])
                        nc.gpsimd.dma_start(out=ct_d[:, uk * 128:(uk + 1) * 128], in_=cseg[:, :])

            # ---- logits phase: wu freed, load Wd ----
            with tc.tile_pool(name="wd", bufs=1) as wdp, \
                 tc.tile_pool(name="lstag", bufs=2) as lstag, \
                 tc.tile_pool(name="lout", bufs=3) as lout, \
                 tc.tile_pool(name="plog", bufs=2, space="PSUM") as plog:
                wd_sb = wdp.tile([128, UK, VS], BF16)
                for uk in range(UK):
                    for hh in range(2):
                        sg = lstag.tile([128, VS // 2], F32, tag="lst")
                        nc.sync.dma_start(out=sg[:, :], in_=wd_d[uk * 128:(uk + 1) * 128, hh * (VS // 2):(hh + 1) * (VS // 2)])
                        nc.vector.tensor_copy(out=wd_sb[:, uk, hh * (VS // 2):(hh + 1) * (VS // 2)], in_=sg[:, :])
                if with_bd:
                    bd_sb = wdp.tile([1, VS], F32)
                    nc.sync.dma_start(out=bd_sb[:, :], in_=_ap(bd_d[:], [[0, 1], [1, VS]]))

                NV = 8
                VC = VS // NV  # 500
                for bb in range(B):
                    for vc in range(NV):
                        lps = plog.tile([128, VC], F32, tag="lps")
                        for uk in range(UK):
                            nc.tensor.matmul(
                                lps[:, :],
                                hsT[:, uk, bb * T:(bb + 1) * T],
                                wd_sb[:, uk, vc * VC:(vc + 1) * VC],
                                start=(uk == 0), stop=(uk == UK - 1))
                        ot = lout.tile([128, VC], F32, tag="ot")
                        if with_bd:
                            bd_bc = _ap(bd_sb[:, vc * VC:(vc + 1) * VC],
                                        [[0, 128], [1, VC]])
                            nc.vector.tensor_add(out=ot[:, :], in0=lps[:, :], in1=bd_bc)
                        else:
                            nc.vector.tensor_copy(out=ot[:, :], in_=lps[:, :])
                        nc.gpsimd.dma_start(out=logits_d[bb, :, vc * VC:(vc + 1) * VC], in_=ot[:, :])
    if not nc.is_finalized():
        nc.finalize()
    return nc


_NC_CACHE = {}


def kernel(initial_input, h0, c0, W, U, b, Wd, bd, rec_masks):
    from concourse.bass_utils import run_bass_kernel_spmd

    with_bias = bool(np.any(b))
    with_bd = bool(np.any(bd))
    key = (with_bias, with_bd)
    if key not in _NC_CACHE:
        _NC_CACHE[key] = build_kernel(T, with_bias, with_bd)
    nc = _NC_CACHE[key]

    in_maps = []
    for c in range(NCORES):
        in_maps.append({
            "x": np.ascontiguousarray(initial_input, np.float32),
            "h0": np.ascontiguousarray(h0, np.float32),
            "c0": np.ascontiguousarray(c0, np.float32),
            "W": np.ascontiguousarray(W, np.float32),
            "U": np.ascontiguousarray(U, np.float32),
            "b": np.ascontiguousarray(b, np.float32),
            "Wd": np.ascontiguousarray(Wd[:, c * VS:(c + 1) * VS], np.float32),
            "bd": np.ascontiguousarray(bd[c * VS:(c + 1) * VS], np.float32),
            "rm": np.ascontiguousarray(rec_masks, np.float32),
        })
    res = run_bass_kernel_spmd(nc, in_maps, core_ids=list(range(NCORES)))
    logits = np.concatenate([res.results[c]["logits"] for c in range(NCORES)], axis=-1)
    return logits, res.results[0]["hT"], res.results[0]["cT"]


# revision 10
# speedup vs baseline: 6368.3200x; 6368.3200x over previous
import sys

if "/opt/trn_rl_repo" not in sys.path:
    sys.path.insert(0, "/opt/trn_rl_repo")

import numpy as np

import concourse.bass as bass
import concourse.tile as tile
from concourse import bacc, mybir
from concourse.masks import make_identity

# Problem shapes (hardcoded per spec)
B, T, D, NU, V = 32, 128, 512, 1024, 32000
G4 = 4 * NU
NCORES = 8
VS = V // NCORES  # vocab shard per core

F32 = mybir.dt.float32
BF16 = mybir.dt.bfloat16

# col-group order (i, f, o, c) -> source gate index in U/b/mask order (i, f, c, o)
CG_SRC = [0, 1, 3, 2]

UK = NU // 128  # 8 u k-tiles
DK = D // 128   # 4 d k-tiles
NCH = 2         # 512-wide halves of the 1024 unit dim


def _ap(apobj, dims, extra_offset=0):
    return bass.AP(tensor=apobj.tensor, offset=apobj.offset + extra_offset, ap=dims)


def build_kernel(t_steps=T, with_bias=False, with_bd=False):
    nc = bacc.Bacc("TRN2", target_bir_lowering=False, debug=False)

    x_d = nc.dram_tensor("x", [B, T, D], F32, kind="ExternalInput")
    h0_d = nc.dram_tensor("h0", [B, NU], F32, kind="ExternalInput")
    c0_d = nc.dram_tensor("c0", [B, NU], F32, kind="ExternalInput")
    w_d = nc.dram_tensor("W", [D, G4], F32, kind="ExternalInput")
    u_d = nc.dram_tensor("U", [NU, G4], F32, kind="ExternalInput")
    b_d = nc.dram_tensor("b", [G4], F32, kind="ExternalInput")
    wd_d = nc.dram_tensor("Wd", [NU, VS], F32, kind="ExternalInput")
    bd_d = nc.dram_tensor("bd", [VS], F32, kind="ExternalInput")
    m_d = nc.dram_tensor("rm", [4, B, NU], F32, kind="ExternalInput")

    logits_d = nc.dram_tensor("logits", [B, T, VS], F32, kind="ExternalOutput")
    ht_d = nc.dram_tensor("hT", [B, NU], F32, kind="ExternalOutput")
    ct_d = nc.dram_tensor("cT", [B, NU], F32, kind="ExternalOutput")

    with tile.TileContext(nc) as tc:
        with tc.tile_pool(name="const", bufs=1) as const:
            # persistent SBUF tensors
            hsT = const.tile([128, UK, B * T], BF16)     # h history, u-land, col = b*T + t
            mT = const.tile([128, UK, 4, B], BF16)       # masks, u-land, cg order
            h0T = const.tile([128, UK, B], BF16)
            idf = const.tile([128, 128], F32)
            idb = const.tile([128, 128], BF16)
            make_identity(nc, idf[:, :])
            make_identity(nc, idb[:, :])
            if with_bias:
                bz = const.tile([1, 4, NU], BF16)
                ones = const.tile([1, B], BF16)
                nc.vector.memset(ones[:, :], 1.0)
            if t_steps < T:
                nc.vector.memset(hsT[:, :, :], 0.0)

            with tc.tile_pool(name="wu", bufs=1) as wu, \
                 tc.tile_pool(name="stag", bufs=2) as stag:
                ppre_cm = tc.tile_pool(name="ppre", bufs=2, space="PSUM")
                ppre = ppre_cm.__enter__()
                u_sb = wu.tile([128, UK, G4], BF16)
                w_sb = wu.tile([128, DK, G4], BF16)

                # ---- load + cast U, W ----
                for uk in range(UK):
                    for hh in range(4):
                        sg = stag.tile([128, 1024], F32, tag="stag")
                        nc.sync.dma_start(out=sg[:, :], in_=u_d[uk * 128:(uk + 1) * 128, hh * 1024:(hh + 1) * 1024])
                        nc.vector.tensor_copy(out=u_sb[:, uk, hh * 1024:(hh + 1) * 1024], in_=sg[:, :])
                for dk in range(DK):
                    for hh in range(4):
                        sg = stag.tile([128, 1024], F32, tag="stag")
                        nc.sync.dma_start(out=sg[:, :], in_=w_d[dk * 128:(dk + 1) * 128, hh * 1024:(hh + 1) * 1024])
                        nc.vector.tensor_copy(out=w_sb[:, dk, hh * 1024:(hh + 1) * 1024], in_=sg[:, :])
                if with_bias:
                    for sg_i in range(4):
                        sg = stag.tile([128, 1024], F32, tag="stag")
                        nc.sync.dma_start(out=sg[0:1, 0:NU], in_=_ap(b_d[:], [[0, 1], [1, NU]], extra_offset=sg_i * NU))
                        g = CG_SRC.index(sg_i)
                        nc.vector.tensor_copy(out=bz[:, g, :], in_=sg[0:1, 0:NU])

                # ---- masks -> u-land bf16 ----
                for g in range(4):
                    sg = stag.tile([128, 1024], F32, tag="stagm")
                    nc.sync.dma_start(out=sg[0:B, 0:NU], in_=m_d[CG_SRC[g], :, :])
                    mb = stag.tile([B, NU], BF16, tag="mb")
                    nc.vector.tensor_copy(out=mb[:, :], in_=sg[0:B, 0:NU])
                    pt = ppre.tile([128, UK, B], BF16, tag="pt")
                    for uk in range(UK):
                        nc.tensor.transpose(pt[:, uk, :], mb[:, uk * 128:(uk + 1) * 128], idb[0:B, 0:B])
                    nc.vector.tensor_copy(out=mT[:, :, g, :], in_=pt[:, :, :])

                # ---- h0 -> u-land bf16 ----
                sg = stag.tile([128, 1024], F32, tag="stagm")
                nc.sync.dma_start(out=sg[0:B, 0:NU], in_=h0_d[:, :])
                hb = stag.tile([B, NU], BF16, tag="mb")
                nc.vector.tensor_copy(out=hb[:, :], in_=sg[0:B, 0:NU])
                pt = ppre.tile([128, UK, B], BF16, tag="pt")
                for uk in range(UK):
                    nc.tensor.transpose(pt[:, uk, :], hb[:, uk * 128:(uk + 1) * 128], idb[0:B, 0:B])
                nc.vector.tensor_copy(out=h0T[:, :, :], in_=pt[:, :, :])

                ppre_cm.__exit__(None, None, None)

                # ---- c0 -> u-land f32 (kept in ping-pong pool) ----
                with tc.tile_pool(name="state", bufs=2) as state, \
                     tc.tile_pool(name="step", bufs=2) as step, \
                     tc.tile_pool(name="xs", bufs=3) as xs, \
                     tc.tile_pool(name="pz", bufs=2, space="PSUM") as pz, \
                     tc.tile_pool(name="pT", bufs=2, space="PSUM") as pT, \
                     tc.tile_pool(name="px", bufs=2, space="PSUM") as px:

                    sgc = stag.tile([128, 1024], F32, tag="stagm")
                    nc.sync.dma_start(out=sgc[0:B, 0:NU], in_=c0_d[:, :])
                    c_cur = state.tile([128, UK, B], F32, tag="c")
                    ptc = px.tile([128, UK, B], F32, tag="ptx")
                    for uk in range(UK):
                        nc.tensor.transpose(ptc[:, uk, :], sgc[0:B, uk * 128:(uk + 1) * 128], idf[0:B, 0:B])
                    nc.vector.tensor_copy(out=c_cur[:, :, :], in_=ptc[:, :, :])

                    # hm for t=0 from h0T
                    hm_cur = step.tile([128, UK, 4 * B], BF16, tag="hm")
                    h0_bc = _ap(h0T[:, :, :], [h0T[:, :, :].ap[0], h0T[:, :, :].ap[1], [0, 4], h0T[:, :, :].ap[2]])
                    nc.vector.tensor_mul(out=hm_cur[:, :, :], in0=h0_bc, in1=mT[:, :, :, :])

                    # x for t=0
                    def emit_x(t):
                        xst = xs.tile([B, D], F32, tag="xst")
                        nc.sync.dma_start(out=xst[:, :], in_=x_d[:, t, :])
                        pxt = px.tile([128, DK, B], F32, tag="ptx")
                        for dk in range(DK):
                            nc.tensor.transpose(pxt[:, dk, :], xst[:, dk * 128:(dk + 1) * 128], idf[0:B, 0:B])
                        xt4 = step.tile([128, DK, B], BF16, tag="xt4")
                        nc.vector.tensor_copy(out=xt4[:, :, :], in_=pxt[:, :, :])
                        return xt4

                    xt4_cur = emit_x(0)

                    hs_full = hsT[:, :, :]

                    for t in range(t_steps):
                        # ---- z matmuls: psum [(4g x 32b), 512] x 2 halves ----
                        zps = [pz.tile([128, 512], F32, tag=f"z{i}", name=f"z{i}") for i in range(NCH)]
                        for nch in range(NCH):
                            for g in range(4):
                                col0 = CG_SRC[g] * NU + nch * 512
                                for dk in range(DK):
                                    nc.tensor.matmul(
                                        zps[nch][32 * g:32 * (g + 1), :],
                                        xt4_cur[:, dk, :],
                                        w_sb[:, dk, col0:col0 + 512],
                                        start=(dk == 0), stop=False,
                                        tile_position=(0, 32 * g))
                                for uk in range(UK):
                                    last = (uk == UK - 1) and not with_bias
                                    nc.tensor.matmul(
                                        zps[nch][32 * g:32 * (g + 1), :],
                                        hm_cur[:, uk, 32 * g:32 * (g + 1)],
                                        u_sb[:, uk, col0:col0 + 512],
                                        start=False, stop=last,
                                        tile_position=(0, 32 * g))
                                if with_bias:
                                    nc.tensor.matmul(
                                        zps[nch][32 * g:32 * (g + 1), :],
                                        ones[0:1, 0:32],
                                        bz[:, CG_SRC[g], nch * 512:(nch + 1) * 512],
                                        start=False, stop=True,
                                        tile_position=(0, 32 * g))

                        # x transposes for next step (fills PE while ACT runs)
                        if t + 1 < t_steps:
                            xt4_nxt = emit_x(t + 1)

                        # ---- gates: sigmoid(i,f,o), copy zc; bf16 (split per half) ----
                        gT = pT.tile([128, UK, 128], BF16, tag="gT")
                        for nch in range(NCH):
                            ghalf = step.tile([128, 512], BF16, tag=f"gates{nch}", name=f"gates{nch}")
                            nc.scalar.activation(out=ghalf[0:96, :],
                                                 in_=zps[nch][0:96, :],
                                                 func=mybir.ActivationFunctionType.Sigmoid)
                            nc.vector.tensor_copy(out=ghalf[96:128, :],
                                                  in_=zps[nch][96:128, :])
                            for uq in range(4):
                                uk = nch * 4 + uq
                                nc.tensor.transpose(gT[:, uk, :], ghalf[:, uq * 128:(uq + 1) * 128], idb[:, :])

                        # ---- cell math in u-land ----
                        tcn = step.tile([128, UK, B], F32, tag="tc", bufs=1)
                        nc.scalar.activation(out=tcn[:, :, :], in_=gT[:, :, 96:128],
                                             func=mybir.ActivationFunctionType.Tanh)
                        fc = step.tile([128, UK, B], F32, tag="fc", bufs=1)
                        nc.vector.tensor_mul(out=fc[:, :, :], in0=gT[:, :, 32:64], in1=c_cur[:, :, :])
                        ic = step.tile([128, UK, B], F32, tag="ic", bufs=1)
                        nc.vector.tensor_mul(out=ic[:, :, :], in0=gT[:, :, 0:32], in1=tcn[:, :, :])
                        c_nxt = state.tile([128, UK, B], F32, tag="c")
                        nc.vector.tensor_add(out=c_nxt[:, :, :], in0=fc[:, :, :], in1=ic[:, :, :])
                        th = step.tile([128, UK, B], F32, tag="th", bufs=1)
                        nc.scalar.activation(out=th[:, :, :], in_=c_nxt[:, :, :],
                                             func=mybir.ActivationFunctionType.Tanh)
                        # h (bf16) directly into history: cols {b*T + t}
                        hdst = _ap(hs_full, [hs_full.ap[0], hs_full.ap[1], [T, B]], extra_offset=t)
                        nc.vector.tensor_mul(out=hdst, in0=gT[:, :, 64:96], in1=th[:, :, :])

                        # hm for next step
                        if t + 1 < t_steps:
                            hm_nxt = step.tile([128, UK, 4 * B], BF16, tag="hm")
                            hsrc = _ap(hs_full, [hs_full.ap[0], hs_full.ap[1], [0, 4], [T, B]], extra_offset=t)
                            nc.vector.tensor_mul(out=hm_nxt[:, :, :], in0=hsrc, in1=mT[:, :, :, :])
                            hm_cur = hm_nxt
                            xt4_cur = xt4_nxt
                        c_cur = c_nxt

                    # ---- final hT (from bf16 h) and cT (f32) back to b-land ----
                    hlast = _ap(hs_full, [hs_full.ap[0], hs_full.ap[1], [T, B]], extra_offset=t_steps - 1)
                    hlast_bf = step.tile([128, UK, B], BF16, tag="hlast")
                    nc.vector.tensor_copy(out=hlast_bf[:, :, :], in_=hlast)
                    for uk in range(UK):
                        pbt = pT.tile([B, 128], BF16, tag="gT", name="pbt")
                        nc.tensor.transpose(pbt[:, :], hlast_bf[:, uk, :], idb[:, :])
                        hseg = step.tile([B, 128], F32, tag="hseg", name="hseg", bufs=2)
                        nc.vector.tensor_copy(out=hseg[:, :], in_=pbt[:, :])
                        nc.gpsimd.dma_start(out=ht_d[:, uk * 128:(uk + 1) * 128], in_=hseg[:, :])
                        pbc = px.tile([B, 128], F32, tag="ptx", name="pbc")
                        cl = step.tile([128, B], F32, tag="cl", bufs=2)
                        nc.vector.tensor_copy(out=cl[:, :], in_=c_cur[:, uk, :])
                        nc.tensor.transpose(pbc[:, :], cl[:, :], idf[:, :])
                        cseg = step.tile([B, 128], F32, tag="cseg", name="cseg", bufs=2)
                        nc.vector.tensor_copy(out=cseg[:, :], in_=pbc[:, :])
                        nc.gpsimd.dma_start(out=ct_d[:, uk * 128:(uk + 1) * 128], in_=cseg[:, :])

            # ---- logits phase: wu freed, load Wd ----
            with tc.tile_pool(name="wd", bufs=1) as wdp, \
                 tc.tile_pool(name="lstag", bufs=2) as lstag, \
                 tc.tile_pool(name="lout", bufs=3) as lout, \
                 tc.tile_pool(name="plog", bufs=2, space="PSUM") as plog:
                wd_sb = wdp.tile([128, UK, VS], BF16)
                for uk in range(UK):
                    for hh in range(2):
                        sg = lstag.tile([128, VS // 2], F32, tag="lst")
                        nc.sync.dma_start(out=sg[:, :], in_=wd_d[uk * 128:(uk + 1) * 128, hh * (VS // 2):(hh + 1) * (VS // 2)])
                        nc.vector.tensor_copy(out=wd_sb[:, uk, hh * (VS // 2):(hh + 1) * (VS // 2)], in_=sg[:, :])
                if with_bd:
                    bd_sb = wdp.tile([1, VS], F32)
                    nc.sync.dma_start(out=bd_sb[:, :], in_=_ap(bd_d[:], [[0, 1], [1, VS]]))

                NV = 8
                VC = VS // NV  # 500
                for bb in range(B):
                    for vc in range(NV):
                        lps = plog.tile([128, VC], F32, tag="lps")
                        for uk in range(UK):
                            nc.tensor.matmul(
                                lps[:, :],
                                hsT[:, uk, bb * T:(bb + 1) * T],
                                wd_sb[:, uk, vc * VC:(vc + 1) * VC],
                                start=(uk == 0), stop=(uk == UK - 1))
                        ot = lout.tile([128, VC], F32, tag="ot")
                        if with_bd:
                            bd_bc = _ap(bd_sb[:, vc * VC:(vc + 1) * VC],
                                        [[0, 128], [1, VC]])
                            nc.vector.tensor_add(out=ot[:, :], in0=lps[:, :], in1=bd_bc)
                        else:
                            nc.vector.tensor_copy(out=ot[:, :], in_=lps[:, :])
                        nc.gpsimd.dma_start(out=logits_d[bb, :, vc * VC:(vc + 1) * VC], in_=ot[:, :])
    if not nc.is_finalized():
        nc.finalize()
    return nc


_NC_CACHE = {}


def kernel(initial_input, h0, c0, W, U, b, Wd, bd, rec_masks):
    from concourse.bass_utils import run_bass_kernel_spmd

    with_bias = bool(np.any(b))
    with_bd = bool(np.any(bd))
    key = (with_bias, with_bd)
    if key not in _NC_CACHE:
        _NC_CACHE[key] = build_kernel(T, with_bias, with_bd)
    nc = _NC_CACHE[key]

    in_maps = []
    for c in range(NCORES):
        in_maps.append({
            "x": np.ascontiguousarray(initial_input, np.float32),
            "h0": np.ascontiguousarray(h0, np.float32),
            "c0": np.ascontiguousarray(c0, np.float32),
            "W": np.ascontiguousarray(W, np.float32),
            "U": np.ascontiguousarray(U, np.float32),
            "b": np.ascontiguousarray(b, np.float32),
            "Wd": np.ascontiguousarray(Wd[:, c * VS:(c + 1) * VS], np.float32),
            "bd": np.ascontiguousarray(bd[c * VS:(c + 1) * VS], np.float32),
            "rm": np.ascontiguousarray(rec_masks, np.float32),
        })
    res = run_bass_kernel_spmd(nc, in_maps, core_ids=list(range(NCORES)))
    logits = np.concatenate([res.results[c]["logits"] for c in range(NCORES)], axis=-1)
    return logits, res.results[0]["hT"], res.results[0]["cT"]
